# revision 11
# baseline (speedup 1.0000x reference)
"""MoE adapter (top-1 of 4 experts, dense all-expert reference) on 8 TRN2 NeuronCores.

Strategy (v3: fp8 DoubleRow, TILE=1024)
---------------------------------------
Data-parallel over the 32768 tokens (4096 per core); expert weights replicated.

Since 4 experts x H=192 = 768, the four expert MLPs stack into two dense
768x768 matmuls.  Both big matmuls run in fp8(e4m3) with DoubleRow perf mode
(256-deep contraction per instruction):

    h    = gelu((512*(x8 @ W1_8) - mask_bias)/512 + b1)  # [768h, tok] fp8
    y64  = h @ W2_8 + one_hot @ (64 b2)                  # [tok, 768] psum
    out  = (y64 + 64 x) / 64                             # /64 on host

Key tricks vs the bf16 baseline:
  * top-1 masking via a -2^20 bias accumulated into the pre-gelu PSUM rows
    of non-selected experts (one tiny K=4 matmul per psum group);
    gelu(-2048) == 0, so no mask multiply / expand anywhere.
  * router is a single fp16 pass (vs hi/lo bf16 = half the matmuls).
  * 1024-token tiles: each mm1 stationary is reused for two 512-col matmuls,
    halving exposed DoubleRow LDWEIGHTS time.
  * weights ride the qAct HWDGE ring ahead of the token-major skip tensor,
    so the PE doesn't stall on them behind tile-0's x loads.
  * scales: x8 = 8x, W*_8 = 64W; gelu scale=1/512 removes mm1's scale;
    skip tensor ships as 64x fp16 so one DVE add drains psum+skip; host
    divides the fp16 output by 64 (exact exponent shift).
"""

import numpy as np
import ml_dtypes

import concourse.bass as bass
import concourse.mybir as mybir
import concourse.tile as tile
from concourse import bacc
from concourse.bass_utils import run_bass_kernel_spmd

BF16 = ml_dtypes.bfloat16
FP16 = np.float16
FP8 = ml_dtypes.float8_e4m3
F32 = np.float32

B, S, D = 16, 2048, 768
H, E = 192, 4
N_CORES = 8
TOK_TOTAL = B * S                 # 32768
TOK = TOK_TOTAL // N_CORES        # 4096 tokens per core
TILE = 1024                       # tokens per pipeline tile
N_TILES = TOK // TILE             # 4
SUBT = TILE // 128                # 8 token subtiles of 128
NTH = TILE // 512                 # 2 512-col halves per tile
KC = D // 128                     # 6 contraction chunks
DC = KC // 2                      # 3 DoubleRow double-chunks

XS = 8.0                          # x fp8 scale
WS = 64.0                         # weight fp8 scale
OS = 64.0                         # output / skip scale
MBIG = float(2 ** 20)             # pre-gelu mask offset (=> -2048 post scale)

_NC_CACHE = None


def _build_bass():
    dt = mybir.dt
    DR = mybir.MatmulPerfMode.DoubleRow
    nc = bacc.Bacc("TRN2", target_bir_lowering=False)

    xf16 = nc.dram_tensor("xf16", [D, TOK], dt.float16, kind="ExternalInput")
    x8 = nc.dram_tensor("x8", [D, TOK], dt.float8e4, kind="ExternalInput")
    xs16 = nc.dram_tensor("xs16", [TOK, D], dt.float16, kind="ExternalInput")
    w18 = nc.dram_tensor("w18", [D, D], dt.float8e4, kind="ExternalInput")
    w28 = nc.dram_tensor("w28", [D, D], dt.float8e4, kind="ExternalInput")
    rw16 = nc.dram_tensor("rw16", [D, E], dt.float16, kind="ExternalInput")
    rbt = nc.dram_tensor("rbt", [32, E], dt.float32, kind="ExternalInput")
    b1r = nc.dram_tensor("b1r", [128, KC], dt.float32, kind="ExternalInput")
    een = nc.dram_tensor("een", [E, 128], dt.bfloat16, kind="ExternalInput")
    b2s = nc.dram_tensor("b2s", [E, D], dt.bfloat16, kind="ExternalInput")
    out = nc.dram_tensor("out", [TOK, D], dt.float16, kind="ExternalOutput")

    xf_r = xf16.rearrange("(c p) t -> p c t", p=128)
    x8_r = x8.rearrange("(c i p) t -> p c i t", i=2, p=128)
    w1_r = w18.rearrange("(c i p) m -> p c i m", i=2, p=128)
    w2_r = w28.rearrange("(c i p) m -> p c i m", i=2, p=128)
    rw_r = rw16.rearrange("(c p) e -> p c e", p=128)
    xs_r = xs16.rearrange("(n p) d -> p n d", p=128)
    out_r = out.rearrange("(n p) d -> p n d", p=128)

    add = mybir.AluOpType.add
    amax = mybir.AluOpType.max
    iseq = mybir.AluOpType.is_equal
    gelu = mybir.ActivationFunctionType.Gelu

    with tile.TileContext(nc) as tc:
        with (
            tc.tile_pool(name="const", bufs=1) as const,
            tc.tile_pool(name="xin", bufs=3) as xin,
            tc.tile_pool(name="hbuf", bufs=2) as hbuf,
            tc.tile_pool(name="obuf", bufs=6) as obuf,
            tc.tile_pool(name="small", bufs=2) as small,
            tc.tile_pool(name="ps_rt", bufs=2, space="PSUM") as ps_rt,
            tc.tile_pool(name="ps_h", bufs=3, space="PSUM") as ps_h,
            tc.tile_pool(name="ps_y5", bufs=2, space="PSUM") as ps_y5,
            tc.tile_pool(name="ps_y2", bufs=1, space="PSUM") as ps_y2,
        ):
            # small constants lead the sync HWDGE ring (tiny, instant) so the
            # router/mask pipeline can start immediately
            rwsb = const.tile([128, KC, E], dt.float16)
            nc.sync.dma_start(rwsb, rw_r)
            rbsb = const.tile([32, E], dt.float32)
            nc.sync.dma_start(rbsb, rbt[:])
            b1sb = const.tile([128, KC], dt.float32)
            nc.sync.dma_start(b1sb, b1r[:])
            eesb = const.tile([E, 128], dt.bfloat16)
            nc.sync.dma_start(eesb, een[:])
            b2sb = const.tile([E, D], dt.bfloat16)
            nc.sync.dma_start(b2sb, b2s[:])
            # weights lead the qAct HWDGE ring (ahead of xst / output stores)
            w1sb = const.tile([128, DC, 2, D], dt.float8e4)
            nc.scalar.dma_start(w1sb, w1_r)
            w2sb = const.tile([128, DC, 2, D], dt.float8e4)
            nc.scalar.dma_start(w2sb, w2_r)

            def load_tiles(it):
                t0 = it * TILE
                xf = xin.tile([128, KC, TILE], dt.float16, tag="xf")
                nc.sync.dma_start(xf, xf_r[:, :, t0 : t0 + TILE])
                x8t = xin.tile([128, DC, 2, TILE], dt.float8e4, tag="x8t")
                nc.sync.dma_start(x8t, x8_r[:, :, :, t0 : t0 + TILE])
                xst = xin.tile([128, SUBT, D], dt.float16, tag="xst")
                nc.scalar.dma_start(xst, xs_r[:, it * SUBT : (it + 1) * SUBT, :])
                return xf, x8t, xst

            def router_pe(xf):
                """fp16 logits^T for one tile -> two psum [4, 512] halves."""
                ps = []
                for th in range(NTH):
                    psrt = ps_rt.tile([E, 512], dt.float32, tag="psrt")
                    for kc in range(KC):
                        nc.tensor.matmul(
                            psrt, rwsb[:, kc, :],
                            xf[:, kc, th * 512 : (th + 1) * 512],
                            start=(kc == 0), stop=(kc == KC - 1),
                        )
                    ps.append(psrt)
                return ps

            def router_mask(ps):
                """one-hot top-1 mask mt [4, TILE] bf16 via 32x32 transposes."""
                G = TILE // 32
                lt32s = small.tile([32, TILE], dt.float32, tag="lt32s")
                for th in range(NTH):
                    nc.scalar.copy(lt32s[0:E, th * 512 : (th + 1) * 512], ps[th])
                # token-major blocks: lt32[p, 32g+r] = lt32s[r, 32g+p]
                lt32 = small.tile([32, TILE], dt.float32, tag="lt32")
                nc.vector.transpose(lt32, lt32s)
                v = lt32.rearrange("p (g r) -> p g r", r=32)
                lt_tok = small.tile([32, G, E], dt.float32, tag="lt_tok")
                nc.vector.tensor_tensor(
                    lt_tok, v[:, :, 0:E],
                    rbsb[:, None, :].to_broadcast((32, G, E)), add,
                )
                mxg = small.tile([32, G], dt.float32, tag="mxg")
                nc.vector.tensor_reduce(
                    out=mxg, in_=lt_tok, axis=mybir.AxisListType.X, op=amax
                )
                mtb = small.tile([32, TILE], dt.bfloat16, tag="mtb")
                mview = mtb.rearrange("p (g r) -> p g r", r=32)
                nc.vector.tensor_tensor(
                    mview[:, :, 0:E], lt_tok,
                    mxg[:, :, None].to_broadcast((32, G, E)), iseq,
                )
                # back-transpose: mt32[e, t] = one_hot[t, e] for e < 4
                mt32 = small.tile([32, TILE], dt.bfloat16, tag="mt32")
                nc.vector.transpose(mt32, mtb)
                return mt32[0:E]

            # PE warm-up burst: spin the HAM up during the DMA head
            dummy = const.tile([128, 512], dt.bfloat16)
            nc.vector.memset(dummy, 0.0)
            psd = ps_h.tile([128, 512], dt.float32, tag="psh")
            for _ in range(10):
                nc.tensor.matmul(psd, dummy[:, 0:128], dummy, start=True, stop=True)

            tiles = {0: load_tiles(0)}
            mt = router_mask(router_pe(tiles[0][0]))
            tiles[1] = load_tiles(1)

            for it in range(N_TILES):
                xf, x8t, xst = tiles[it]
                if it + 2 < N_TILES:
                    tiles[it + 2] = load_tiles(it + 2)

                # ---- mm1 (fp8 DoubleRow) + mask-bias + gelu -> mh fp8 ----
                # stationary (dc,hc) reused across both 512-token halves.
                mh = hbuf.tile([128, KC, TILE], dt.float8e4, tag="mh")
                for hc in range(KC):
                    psh = [ps_h.tile([128, 512], dt.float32, tag="psh",
                                     name=f"psh_{it}_{hc}_{th}")
                           for th in range(NTH)]
                    for th in range(NTH):
                        # -2^20 into non-selected experts' rows (gelu -> 0)
                        nc.tensor.matmul(
                            psh[th], eesb, mt[:, th * 512 : (th + 1) * 512],
                            start=True, stop=False,
                        )
                    for dc in range(DC):
                        for th in range(NTH):
                            nc.tensor.matmul(
                                psh[th],
                                w1sb[:, dc, :, hc * 128 : (hc + 1) * 128],
                                x8t[:, dc, :, th * 512 : (th + 1) * 512],
                                start=False, stop=(dc == DC - 1),
                                perf_mode=DR,
                            )
                    for th in range(NTH):
                        nc.scalar.activation(
                            mh[:, hc, th * 512 : (th + 1) * 512], psh[th], gelu,
                            bias=b1sb[:, hc : hc + 1], scale=1.0 / (XS * WS),
                        )

                # router for tile n+1 hides its DVE chain under mm2
                ps_n = router_pe(tiles[it + 1][0]) if it + 1 < N_TILES else None

                # ---- mm2 (fp8 DoubleRow) + b2 + skip-add, token-major ----
                # the two in-flight [128,256] psums share one bank (slices)
                psy2d = ps_y2.tile([128, 2, 256], dt.float32, tag="psy2d")
                for a in range(SUBT):
                    osb = obuf.tile([128, D], dt.float16, tag="osb")
                    psy5 = ps_y5.tile([128, 512], dt.float32, tag="psy5")
                    psy2 = psy2d[:, a % 2, :]
                    mta = mt[:, a * 128 : (a + 1) * 128]
                    nc.tensor.matmul(psy5, mta, b2sb[:, 0:512], start=True, stop=False)
                    nc.tensor.matmul(psy2, mta, b2sb[:, 512:768], start=True, stop=False)
                    for dc in range(DC):
                        mha = mh[:, 2 * dc : 2 * dc + 2, a * 128 : (a + 1) * 128]
                        nc.tensor.matmul(
                            psy5, mha, w2sb[:, dc, :, 0:512],
                            start=False, stop=(dc == DC - 1), perf_mode=DR,
                        )
                        nc.tensor.matmul(
                            psy2, mha, w2sb[:, dc, :, 512:768],
                            start=False, stop=(dc == DC - 1), perf_mode=DR,
                        )
                    nc.vector.tensor_tensor(osb[:, 0:512], psy5, xst[:, a, 0:512], add)
                    nc.vector.tensor_tensor(osb[:, 512:768], psy2, xst[:, a, 512:768], add)
                    nc.scalar.dma_start(out_r[:, it * SUBT + a, :], osb)

                if ps_n is not None:
                    mt = router_mask(ps_n)
                del tiles[it]

    nc.compile()
    return nc


def _prep_inputs(x, router_w, router_b, w1, b1, w2, b2):
    """Host-side packing: cast/scale/transpose; returns per-core input dicts."""
    xf = np.ascontiguousarray(np.asarray(x, dtype=F32).reshape(TOK_TOTAL, D))

    rw = np.asarray(router_w, dtype=F32).astype(FP16)         # [D, E]
    rb = np.ascontiguousarray(
        np.tile(np.asarray(router_b, dtype=F32).reshape(1, E), (32, 1))
    )

    w1f = np.asarray(w1, dtype=F32)                           # [E, D, H]
    w2f = np.asarray(w2, dtype=F32)                           # [E, H, D]
    b1f = np.asarray(b1, dtype=F32)                           # [E, H]
    b2f = np.asarray(b2, dtype=F32)                           # [E, D]

    # experts interleaved along the stacked hidden dim (unit j of expert e at
    # index 4j + e) so the mask pattern repeats identically per 128-row chunk.
    w1s = np.ascontiguousarray(w1f.transpose(1, 2, 0).reshape(D, H * E))
    w2s = np.ascontiguousarray(w2f.transpose(1, 0, 2).reshape(H * E, D))
    w18 = (WS * w1s).astype(FP8)
    w28 = (WS * w2s).astype(FP8)
    b1all = np.ascontiguousarray(b1f.T.reshape(E * H))
    b1r = np.ascontiguousarray(b1all.reshape(KC, 128).T).astype(F32)
    b2sb = (OS * b2f).astype(BF16)

    een = np.full((E, 128), -MBIG, dtype=BF16)
    for e in range(E):
        een[e, e::E] = 0.0

    in_maps = []
    for c in range(N_CORES):
        sl = slice(c * TOK, (c + 1) * TOK)
        xc = xf[sl]
        xT = np.ascontiguousarray(xc.T)
        in_maps.append(
            {
                "xf16": xT.astype(FP16),
                "x8": (XS * xT).astype(FP8),
                "xs16": (OS * xc).astype(FP16),
                "w18": w18,
                "w28": w28,
                "rw16": rw,
                "rbt": rb,
                "b1r": b1r,
                "een": een,
                "b2s": b2sb,
            }
        )
    return in_maps


def _get_nc():
    global _NC_CACHE
    if _NC_CACHE is None:
        _NC_CACHE = _build_bass()
    return _NC_CACHE


def kernel(x, router_w, router_b, w1, b1, w2, b2, _trace=False, _trace_kwargs=None):
    in_maps = _prep_inputs(x, router_w, router_b, w1, b1, w2, b2)
    nc = _get_nc()
    res = run_bass_kernel_spmd(
        nc,
        in_maps,
        core_ids=list(range(N_CORES)),
        trace=_trace,
        **(_trace_kwargs or {}),
    )
    outs = [np.asarray(r["out"], dtype=F32) * (1.0 / OS) for r in res.results]
    full = np.concatenate(outs, axis=0).reshape(B, S, D)
    if _trace:
        kernel.last_results = res
    return full
